# revision 85
# baseline (speedup 1.0000x reference)
"""AdaptivePCEN Trainium2 kernel.

Data-parallel over batch: core i computes batches [4i, 4i+4) of the
[32, 128, 4000] input. PPN weights replicated. Per core:
  - PE (fp8 e4m3, DoubleRow perf mode): each matmul contracts K=256 in
    one pass at 2 cols/cycle. The (Xprev, Xcur) pair arrives from the
    host as an interleaved [F, 2, T] fp8 buffer (an overlapping stride-1
    AP over one copy wedges the device); the gate matmuls read a
    combined [h1|h2] fp8 tile the relu evacuations fill. Phase-B element
    math keeps a separate bf16 copy of X (fp8 X is only matmul fodder;
    X enters the output linearly so 8-bit would cost too much accuracy
    there).
  - ACT: tanh/exp gate evacuations + PCEN epilogue (no Softplus LUT on
    this toolchain: softplus = ln(1+exp(z))). p1/p2 exps fused into one
    double-width instruction per chunk.
  - DVE: tensor_tensor_scan runs the EMA recurrence M_t = (1-s)M + s X
    along the free (time) axis, chained across chunks via the
    per-partition carry.
  - GpSimd (otherwise idle) takes the final f32 subtract.
Matmul accumulation groups must stay inside one 2KB PSUM bank
(bank-crossing output corrupts); every DoubleRow matmul writes 512
f32 cols = exactly one bank.
Phase-B chunk edges are per-batch: batch 0 starts with a small chunk
so the epilogue engines ramp early; the last batch ends with a small
chunk to shorten the serial drain tail.
"""

import numpy as np

B, F, T, H = 32, 128, 4000, 256
N_CORES = 8
BSH = B // N_CORES  # batches per core
CHA = 1024  # phase-A chunk (2 psum banks; subs at 0/512 bank-aligned)
SUBA = 512

_COMPILED = {}


def _chunks(t, ch):
    out = []
    t0 = 0
    while t0 < t:
        out.append((t0, min(ch, t - t0)))
        t0 += ch
    return out


def _build(bsh=BSH, t=T, cha=CHA, suba=SUBA):
    from contextlib import ExitStack

    import concourse.tile as tile
    from concourse import bacc, mybir
    from concourse.tile_rust import add_dep_helper

    f32 = mybir.dt.float32
    bf16 = mybir.dt.bfloat16
    fp8 = mybir.dt.float8e4
    AF = mybir.ActivationFunctionType
    OP = mybir.AluOpType
    DR = mybir.MatmulPerfMode.DoubleRow
    EPS = 1e-6

    nc = bacc.Bacc(
        "TRN2", target_bir_lowering=False, debug=False, num_devices=N_CORES
    )

    # X8: fp8 matmul copy, host-interleaved [F, 2, t]: first t cols are
    # Xprev (X shifted right by one, edge-padded), next t cols are X
    X8 = nc.dram_tensor("X8", [bsh * F, 2 * t], fp8, kind="ExternalInput").ap()
    # XB: plain bf16 copy for the phase-B element math
    XB = nc.dram_tensor("XB", [bsh * F, t], bf16, kind="ExternalInput").ap()
    # weights pre-packed on the host in DoubleRow layout: partition k holds
    # [W[k, :] | W[128+k, :]] so lhsT views are [K=128, 2, M]
    W1 = nc.dram_tensor("W1", [F, 2 * H], fp8, kind="ExternalInput").ap()
    b1 = nc.dram_tensor("b1", [F, 2], f32, kind="ExternalInput").ap()
    W2 = nc.dram_tensor("W2", [F, 8 * F], fp8, kind="ExternalInput").ap()
    b2 = nc.dram_tensor("b2", [F, 4], f32, kind="ExternalInput").ap()
    out = nc.dram_tensor("out", [bsh * F, t], f32, kind="ExternalOutput").ap()

    def phb_edges_for(b):
        # 2 chunks for every batch: both finer (3-chunk tails) and coarser
        # splits measured slower end-to-end
        return [(0, 2 * cha), (2 * cha, t - 2 * cha)]

    tbmax = 2 * cha

    with tile.TileContext(nc) as tc, ExitStack() as ctx:
        const = ctx.enter_context(tc.tile_pool(name="const", bufs=1))
        xpool = ctx.enter_context(tc.tile_pool(name="xpool", bufs=2))
        # [512] h tiles, single-buffered + gpsum bufs=2 (6 of 8 banks):
        # counterintuitively the FASTEST allocation -- both gpsum bufs=3
        # and hpsum bufs=2 measured ~10us slower (the tight h ping-pong
        # paces PE in a way the scheduler's static orders depend on)
        hpsum = ctx.enter_context(tc.tile_pool(name="hpsum", bufs=1, space="PSUM"))
        gpsum = ctx.enter_context(tc.tile_pool(name="gpsum", bufs=2, space="PSUM"))
        hsb = ctx.enter_context(tc.tile_pool(name="hsb", bufs=2))
        gates = ctx.enter_context(tc.tile_pool(name="gates", bufs=2))
        tmp = ctx.enter_context(tc.tile_pool(name="tmp", bufs=1))

        # ---- startup DMA order: only w1 + bias1 gate the first h matmul
        # and its relu evac; w2/b2 are needed ~2.5us later (first gate
        # matmul / first tanh evac), so they queue after the first X
        # pieces.  bias1 must stay early (moving it late cost 3.3us). ----
        w1 = const.tile([F, 2 * H], fp8, tag="w1")
        nc.sync.dma_start(out=w1[:], in_=W1[:])
        w1v = w1[:].rearrange("p (i m) -> p i m", i=2)  # [F, 2, 2H...256]

        bias1 = const.tile([F, 2], f32, tag="bias1")
        nc.sync.dma_start(out=bias1[:], in_=b1[:])

        w2 = const.tile([F, 8 * F], fp8, tag="w2")
        w2v = w2[:].rearrange("p (i m) -> p i m", i=2)  # [F, 2, 512]
        bias2 = const.tile([F, 4], f32, tag="bias2")
        epsb = const.tile([F, 1], f32, tag="epsb")
        nc.vector.memset(epsb[:], EPS)
        bias2h = const.tile([F, 4], f32, tag="bias2h")

        def load_w2():
            nc.sync.dma_start(out=w2[:], in_=W2[:])
            nc.sync.dma_start(out=bias2[:], in_=b2[:])
            nc.vector.tensor_scalar(bias2h[:], bias2[:], 0.5, None, OP.mult)

        def load_x(b):
            # xbuf: [F, 2*t] fp8 interleaved (prev | cur); contiguous 2D
            # pieces (strided 3D DMAs cost ~2x in descriptor generation),
            # piece 1 of each half covers phase-A chunk 0 for a fast start
            xbuf = xpool.tile([F, 2 * t], fp8, tag="xbuf", name=f"xbuf_{b}")
            r = slice(b * F, (b + 1) * F)
            nc.sync.dma_start(out=xbuf[:, 0:cha], in_=X8[r, 0:cha])
            nc.sync.dma_start(out=xbuf[:, t : t + cha], in_=X8[r, t : t + cha])
            if b == 0:
                load_w2()
            # xbb is read until late in phase B, so triple-buffer it: with
            # bufs=2 the b+2 input prefetch stalls behind b's last reader
            xbb = xpool.tile([F, t], bf16, tag="xbb", name=f"xbb_{b}", bufs=3)
            nc.sync.dma_start(out=xbb[:, 0:cha], in_=XB[r, 0:cha])
            nc.sync.dma_start(out=xbuf[:, cha:t], in_=X8[r, cha:t])
            nc.sync.dma_start(
                out=xbuf[:, t + cha : 2 * t], in_=X8[r, t + cha : 2 * t]
            )
            nc.sync.dma_start(out=xbb[:, cha:t], in_=XB[r, cha:t])
            return xbuf[:].rearrange("p (i m) -> p i m", i=2), xbb

        cur_x = load_x(0)

        prev_act = [None]  # last ACT inst of previous batch's chain
        NL_SET = 6  # natural_log_exp_and_others in act_info.json

        prev_p12 = [[]]  # previous batch's deferred p12 exps

        def phase_a(b, xv):
            phb_edges = phb_edges_for(b)
            # batch 0 ramps with two 512-wide chunks: its sigmoid block has
            # no deferred-exp filler, so the first tanh's latency is fully
            # exposed -- halving the first chunk starts the pipe earlier
            if b == 0:
                a_chunks = [(0, 512), (512, 512)] + [
                    (o + 1024, w) for o, w in _chunks(t - 1024, cha)
                ]
            else:
                a_chunks = _chunks(t, cha)
            evacs = []
            # gate tiles split per phase-B chunk so chunk 0's scan prep can
            # start while phase A is still filling chunk 1's tiles
            gt = []
            for j, (off, w) in enumerate(phb_edges):
                q = j % 2
                gt.append({
                    "s": gates.tile([F, w], bf16, tag=f"s{q}",
                                    name=f"s_{b}_{j}"),
                    "al": gates.tile([F, w], bf16, tag=f"al{q}",
                                     name=f"al_{b}_{j}"),
                    "r": gates.tile([F, w], bf16, tag=f"r{q}",
                                    name=f"r_{b}_{j}"),
                })
            E_sb = gates.tile([F, t], bf16, tag="E", name=f"E_{b}")

            sig_insts = []

            for t0, cw in a_chunks:
                hs = hsb.tile([F, 2 * cha], fp8, tag="hs")
                hsv = hs[:].rearrange("p (i m) -> p i m", i=2)
                # h psum per 512-sub (1 bank each, double-buffered): keeps
                # the h matmul -> relu evac pipe fine-grained so batch b+1's
                # matmuls never wait on a whole-chunk drain
                for s0, sw_ in _chunks(cw, suba):
                    hp1 = hpsum.tile([F, suba], f32, tag="h1")
                    hp2 = hpsum.tile([F, suba], f32, tag="h2")
                    xr = xv[:, :, t0 + s0 : t0 + s0 + sw_]
                    sl = slice(0, sw_)
                    nc.tensor.matmul(hp1[:, sl], w1v[:, :, 0:F], xr,
                                     start=True, stop=True, perf_mode=DR)
                    nc.tensor.matmul(hp2[:, sl], w1v[:, :, F:H], xr,
                                     start=True, stop=True, perf_mode=DR)
                    evacs.append(nc.vector.tensor_scalar(
                        hs[:, s0 : s0 + sw_], hp1[:, sl], bias1[:, 0:1], 0.0,
                        OP.add, OP.max,
                    ))
                    evacs.append(nc.vector.tensor_scalar(
                        hs[:, cha + s0 : cha + s0 + sw_], hp2[:, sl],
                        bias1[:, 1:2], 0.0, OP.add, OP.max,
                    ))

                j = next(
                    i for i, (off, w) in enumerate(phb_edges)
                    if off <= t0 < off + w
                )
                joff = t0 - phb_edges[j][0]
                # gates sequentially: u=tanh((z+b)/2) for s/alpha/r and
                # E=exp(z+b) for delta -- all four in exp_and_others
                for g, key in ((0, "s"), (1, "al"), (3, "r"), (2, "E")):
                    dest = E_sb if g == 2 else gt[j][key]
                    gp = gpsum.tile([F, cha], f32, tag="g")
                    for s0, sw_ in _chunks(cw, suba):
                        sl = slice(s0, s0 + sw_)
                        nc.tensor.matmul(
                            gp[:, sl], w2v[:, :, g * F : (g + 1) * F],
                            hsv[:, :, s0 : s0 + sw_],
                            start=True, stop=True, perf_mode=DR,
                        )
                    if g == 2:
                        sig_insts.append(
                            nc.scalar.activation(
                                dest[:, t0 : t0 + cw], gp[:, 0:cw],
                                AF.Exp, bias=bias2[:, 2:3],
                            )
                        )
                    else:
                        sig_insts.append(
                            nc.scalar.activation(
                                dest[:, joff : joff + cw], gp[:, 0:cw],
                                AF.Tanh, bias=bias2h[:, g : g + 1], scale=0.5,
                            )
                        )
            return {"gt": gt, "E_sb": E_sb, "sig": sig_insts,
                    "edges": phb_edges, "evacs": evacs}

        def phase_b(b, st, xbb, next_evacs):
            gt, E_sb, sig_insts = st["gt"], st["E_sb"], st["sig"]
            phb_edges = st["edges"]
            # ACT order per batch: [sigmoid evacs, with the PREVIOUS
            # batch's deferred p12 exps spliced in as filler -- Exp lives
            # in the tanh table set too, so this costs no table load and
            # soaks up the waits on PE's gate matmuls] ->
            # LoadActFuncSet(nl_exp) -> grouped ln/exp epilogue.
            act_chain = list(sig_insts)
            if prev_p12[0]:
                # insert at original sig indices 10, 7, 3, 0 (reverse order
                # keeps the positions stable while inserting)
                for pos, inst in zip((10, 7, 3, 0), reversed(prev_p12[0])):
                    act_chain[pos:pos] = [inst]
                prev_p12[0] = []
            ld_inst = nc.scalar.add_instruction(
                mybir.InstLoadActFuncSet(
                    name=nc.get_next_instruction_name(),
                    act_func_set_id=NL_SET,
                    ins=[],
                    outs=[],
                )
            )
            act_chain.append(ld_inst)

            # ---- phase B: per-chunk, pipelined ----
            # delta path chunked at the phb edge: dl1/ld1 slot between L0
            # and L1 in the ACT chain, buying chunk-1's scan ~7us of slack
            dl_f = tmp.tile([F, t], bf16, tag="DL", name=f"dl_{b}")
            ld_f = tmp.tile([F, t], bf16, tag="LD", name=f"ld_{b}")
            dlld = []
            for off, w in phb_edges:
                cs = slice(off, off + w)
                i_dl = nc.scalar.activation(dl_f[:, cs], E_sb[:, cs],
                                            AF.Ln, bias=1.0)
                i_ld = nc.scalar.activation(ld_f[:, cs], dl_f[:, cs], AF.Ln)
                dlld.append((i_dl, i_ld))

            chunk_insts = []
            dv = []  # per-chunk dicts of DVE insts for the explicit chain
            carry = None
            for k, (off, w) in enumerate(phb_edges):
                q = k % 2  # alternating temp slots (no cross-chunk waits)
                gte = gt[k]

                names = iter(range(1000))

                def tl(slot, dt=bf16, wide=1):
                    return tmp.tile(
                        [F, wide * tbmax], dt, tag=f"{slot}{q}",
                        name=f"phb_{b}_{k}_{slot}{q}_{next(names)}",
                    )

                cs = slice(off, off + w)
                xck = xbb[:, off : off + w]
                sw = slice(0, w)

                dl = dl_f[:, cs]
                ld = ld_f[:, cs]

                c = {}
                # tanh halves -> real gates, in place per chunk
                # (tensor_scalar runs 4x; scalar_tensor_tensor would be 1x)
                a_sb = tl("R")  # a = 1-s = 0.5 - 0.5u, straight from u
                c["a"] = nc.vector.tensor_scalar(
                    a_sb[:, sw], gte["s"][:], -0.5, 0.5, OP.mult, OP.add
                )
                c["s"] = nc.vector.tensor_scalar(
                    gte["s"][:], gte["s"][:], 0.5, 0.5, OP.mult, OP.add
                )
                c["al"] = nc.vector.tensor_scalar(
                    gte["al"][:], gte["al"][:], 0.5, 0.5, OP.mult, OP.add
                )
                c["r"] = nc.vector.tensor_scalar(
                    gte["r"][:], gte["r"][:], 0.5, 0.5, OP.mult, OP.add
                )
                # bb stays on DVE: gpsimd (2x slower, queue shared with
                # ob + output DMAs) ahead of the scan measured 27us worse
                bb = tl("S")
                c["bb"] = nc.vector.tensor_tensor(
                    bb[:, sw], gte["s"][:], xck, OP.mult
                )

                M = tl("M", f32)
                c["scan"] = nc.vector.tensor_tensor_scan(
                    M[:, sw], a_sb[:, sw], bb[:, sw],
                    carry if carry is not None else 0.0,
                    OP.mult, OP.add,
                )
                carry = M[:, w - 1 : w]

                L = tl("R")  # a freed after scan; bf16 so t1 gets DVE 2x
                i_L = nc.scalar.activation(L[:, sw], M[:, sw], AF.Ln, bias=epsb[:])
                t1 = tl("S")  # bb freed after scan
                c["t1"] = nc.vector.tensor_tensor(
                    t1[:, sw], gte["al"][:], L[:, sw], OP.mult
                )
                e1 = tl("P")
                i_e1 = nc.scalar.activation(e1[:, sw], t1[:, sw], AF.Exp, scale=-1.0)
                num = tl("R")  # L freed after t1
                c["num"] = nc.vector.tensor_tensor(
                    num[:, sw], xck, e1[:, sw], OP.mult
                )
                base = tl("S")  # t1 freed after e1
                c["base"] = nc.vector.tensor_tensor(
                    base[:, sw], num[:, sw], dl, OP.add
                )
                lb = tl("P")  # e1 freed after num
                i_lb = nc.scalar.activation(lb[:, sw], base[:, sw], AF.Ln)
                # t2 = r*lb and t3 = r*ld side by side, one fused exp.
                # R slot again (num freed after base): its f32 sizing is
                # exactly 2*tbmax bf16, so this costs no extra SBUF
                t23 = tl("R", bf16, wide=2)
                # both products stay on DVE: moving them to gpsimd (2x
                # slower, in-order queue shared with ob+output DMAs)
                # measured 18us WORSE despite unloading DVE's hot window
                c["t23a"] = nc.vector.tensor_tensor(
                    t23[:, 0:w], gte["r"][:], lb[:, sw], OP.mult
                )
                c["t23b"] = nc.vector.tensor_tensor(
                    t23[:, w : 2 * w], gte["r"][:], ld, OP.mult
                )
                # two separate exps (not one fused 2w instr): they are
                # deferred into the next sigmoid block as filler, and four
                # smaller pieces pack the PE-wait gaps better than two
                p12 = tl("F2", f32, wide=2)
                i_p2 = nc.scalar.activation(
                    p12[:, w : w + w], t23[:, w : w + w], AF.Exp
                )
                i_p1 = nc.scalar.activation(p12[:, 0:w], t23[:, 0:w], AF.Exp)
                # final subtract on the (otherwise idle) gpsimd engine
                # (NOT into the M slot: the next chunk's scan still needs
                # the carry column, and the WAR would serialize the scans).
                # The very last chunk runs it on DVE instead -- gpsimd is
                # ~2x slower per element and sits on the drain tail --
                # and in two halves so the first half's output DMA overlaps
                # the second half's subtract.
                ob = tl("R", f32)  # t23 freed after p12
                ro = out[b * F : (b + 1) * F]
                if b == bsh - 1 and k == len(phb_edges) - 1:
                    hw_ = w // 2
                    nc.vector.tensor_tensor(
                        ob[:, 0:hw_], p12[:, 0:hw_], p12[:, w : w + hw_],
                        OP.subtract,
                    )
                    nc.gpsimd.dma_start(
                        out=ro[:, off : off + hw_], in_=ob[:, 0:hw_]
                    )
                    nc.vector.tensor_tensor(
                        ob[:, hw_:w], p12[:, hw_:w], p12[:, w + hw_ : 2 * w],
                        OP.subtract,
                    )
                    nc.gpsimd.dma_start(
                        out=ro[:, off + hw_ : off + w], in_=ob[:, hw_:w]
                    )
                else:
                    nc.gpsimd.tensor_tensor(
                        ob[:, sw], p12[:, 0:w], p12[:, w : 2 * w],
                        OP.subtract,
                    )
                    # output DMA issued from the Pool engine: keeps the
                    # sync queue free for input prefetches (Pool DGE
                    # dispatch is also far cheaper than SP's)
                    nc.gpsimd.dma_start(out=ro[:, cs], in_=ob[:, sw])
                chunk_insts.append((i_L, i_e1, i_lb, i_p1, i_p2))
                dv.append(c)

            # ---- explicit DVE order: without it the scheduler happily
            # DVE nudges: chunk-1's scan ahead of chunk-0's num, then the
            # num->base pairs back-to-back -- the trace shows lb0/lb1
            # stalling ~3us because the scheduler wedges next-batch relu
            # evacuations between e1 and base.  (Wider explicit DVE chains
            # all measured worse; this pins only the lb-feeding tail.)
            mid = [dv[1]["scan"], dv[0]["num"], dv[0]["base"],
                   dv[1]["num"], dv[1]["base"]]
            for prv, nxt in zip(mid, mid[1:]):
                add_dep_helper(nxt.ins, prv.ins, sync=False,
                               reason="num-base priority")

            # ACT chain: per-func groups across chunks (all nl_exp set;
            # order is for pipelining only).  p12 exps are NOT placed here:
            # they ride in the next batch's sigmoid block (same table),
            # where ACT otherwise waits on PE's gate matmuls.
            ci = chunk_insts
            act_chain.extend([dlld[0][0], dlld[0][1], ci[0][0],
                              dlld[1][0], dlld[1][1], ci[1][0]])
            for idx in range(1, 3):
                act_chain.extend([ci[0][idx], ci[1][idx]])
            if b == bsh - 1:
                act_chain.extend([ci[0][3], ci[0][4], ci[1][3], ci[1][4]])
            else:
                prev_p12[0] = [ci[0][3], ci[0][4], ci[1][3], ci[1][4]]
            if prev_act[0] is not None:
                add_dep_helper(
                    act_chain[0].ins, prev_act[0].ins, sync=False,
                    reason="batch act order",
                )
            for prv, nxt in zip(act_chain, act_chain[1:]):
                add_dep_helper(nxt.ins, prv.ins, sync=False, reason="act order")
            prev_act[0] = act_chain[-1]

        # software-pipelined emission: batch b+1's phase A (and its input
        # prefetch) is emitted BEFORE batch b's phase B, so every engine's
        # static order interleaves the two batches instead of serializing
        # a full drain at each batch boundary
        st_cur = phase_a(0, cur_x[0])
        x_cur = cur_x
        for b in range(bsh):
            if b + 1 < bsh:
                x_nxt = load_x(b + 1)
                st_nxt = phase_a(b + 1, x_nxt[0])
                phase_b(b, st_cur, x_cur[1], st_nxt["evacs"])
                st_cur, x_cur = st_nxt, x_nxt
            else:
                phase_b(b, st_cur, x_cur[1], [])

    nc.compile()
    return nc


def _get(key=(BSH, T, CHA, SUBA)):
    if key not in _COMPILED:
        _COMPILED[key] = _build(*key)
    return _COMPILED[key]


def _in_maps(X, W1, b1, W2, b2):
    import ml_dtypes

    bf = ml_dtypes.bfloat16
    f8 = ml_dtypes.float8_e4m3
    # DoubleRow pack: partition k holds [W[k, :] | W[128+k, :]]
    w1p = np.ascontiguousarray(
        W1.reshape(2, F, H).transpose(1, 0, 2).reshape(F, 2 * H).astype(f8)
    )
    w2p = np.ascontiguousarray(
        W2.reshape(2, F, 4 * F).transpose(1, 0, 2).reshape(F, 8 * F).astype(f8)
    )
    b1p = np.ascontiguousarray(b1.reshape(2, F).T.astype(np.float32))
    b2p = np.ascontiguousarray(b2.reshape(4, F).T.astype(np.float32))
    Xb = X.reshape(B * F, T)
    X8 = Xb.astype(f8)
    Xl = np.empty((B * F, 2 * T), dtype=f8)
    Xl[:, 1 : T] = X8[:, : T - 1]  # prev half (shifted right by one)
    Xl[:, 0] = X8[:, 0]            # edge: X_prev[0] = X[0]
    Xl[:, T :] = X8                # cur half
    Xbb = Xb.astype(bf)
    maps = []
    for i in range(N_CORES):
        maps.append(
            {
                "X8": np.ascontiguousarray(Xl[i * BSH * F : (i + 1) * BSH * F]),
                "XB": np.ascontiguousarray(Xbb[i * BSH * F : (i + 1) * BSH * F]),
                "W1": w1p,
                "b1": b1p,
                "W2": w2p,
                "b2": b2p,
            }
        )
    return maps


def run(X, W1, b1, W2, b2, trace=False, **kw):
    from concourse.bass_utils import run_bass_kernel_spmd

    nc = _get()
    res = run_bass_kernel_spmd(
        nc,
        _in_maps(X, W1, b1, W2, b2),
        core_ids=list(range(N_CORES)),
        trace=trace,
        **kw,
    )
    out = np.concatenate(
        [res.results[i]["out"].reshape(BSH, F, T) for i in range(N_CORES)],
        axis=0,
    ).astype(np.float32)
    return out, res


def kernel(X, W1, b1, W2, b2):
    return run(X, W1, b1, W2, b2)[0]
